# revision 21
# baseline (speedup 1.0000x reference)
"""Trainium2 Bass kernel for nn_DiverseRegDCConv2d.

Per-sample dynamic 3x3 conv: filters are generated per sample from an
8-column weight bank (wgen[b] = se[b] @ bank.T), then applied as a
standard 256->256 conv on 28x28 with padding 1.

Sharding (8 cores): 4 batch-groups x 2 out-channel halves. Each core
handles 8 samples x 128 out channels; no cross-device communication.

On-device filter generation: the bank half is pre-arranged on the host
into 128x128 stationary tiles whose partition axis is (n, g) with
n = bank column (8) and g = 16 different (k, o)-blocks; the streaming
operand is a block-diagonal arrangement of inputs_se. One matmul
produces filters for 16 (k,o)-pairs x 8 samples with the conv's
contraction axis (input channel) on PSUM partitions -- exactly the
lhsT layout the conv matmuls need.

Schedule (v2):
 - PE warm-up: a short chain of dummy matmuls on a zeroed SBUF tile
   keeps the tensor engine busy (and its p-state ramp running) while
   the first weight block is still in flight from HBM.
 - Weight bank blocks are stored pre-transposed in DRAM so each DMA
   descriptor moves 2048 contiguous bytes (full DMA bandwidth; 256B
   descriptors run at half rate).
 - Conv accumulates all 18 (cc, k) contributions of an output tile in
   a single PSUM group; evacuation is one scalar-engine activation
   (+bias, fp16 out) straight to the store tile. No cc-partial
   parking, no vector-engine adds.
 - DMA issue is spread across queues: SP streams the weight bank,
   the scalar engine's HWDGE queue loads x, and output stores go
   through the Pool engine's SWDGE queue (the last stores use SP to
   shorten the end-of-kernel chain).
 - During the weight stream, NPROG samples' output tiles accumulate
   progressively (lagging the stream by LAG blocks); the remaining
   samples run as dense 18-matmul bursts once all filters exist.

Precision: fp16 operands with fp32 PSUM accumulation; output stored
as fp16 and upcast on host. End-to-end relative error vs the fp32
reference is ~1e-3.
"""

import sys

for _p in ("/opt/trn_rl_repo", "/root/.axon_site/_ro/trn_rl_repo"):
    if _p not in sys.path:
        sys.path.append(_p)

import numpy as np

import concourse.bass as bass
import concourse.mybir as mybir
from concourse import bacc
from concourse.bass_utils import run_bass_kernel_spmd
from concourse.tile import TileContext

B, C, O, KS, H, W, NUM = 32, 256, 256, 3, 28, 28, 8
P = 128
NCORES = 8
BG, OHALF = 4, 2          # batch-groups x out-channel halves
S = B // BG               # samples per core = 8
OC = O // OHALF           # out channels per core = 128
CC = C // P               # input-channel chunks = 2
G = 16                    # (k,o)-blocks per wgen matmul (with NUM=8 fills K=128)
NP = KS * KS * OC         # (k, o_local) pairs per c-chunk = 1152
NM = NP // G              # wgen matmuls per c-chunk = 72
NB = CC * KS * KS         # weight-bank blocks (8 tiles each) = 18
F32 = mybir.dt.float32
F16 = mybir.dt.float16

HH = H // 2               # 14 output rows per conv matmul -> N = 392
NPROG = 3                 # samples that accumulate during the weight stream
LAG = 2                   # conv trails the wgen stream by this many blocks
JOIN = (0, 1, 2)          # conv-block at which prog sample s joins
WARM_PRE = 28             # PE warm-up matmuls before the first wgen block
FILLS = (8, 4)            # idle-filler matmuls after wgen blocks 0, 1
NSP_STORES = 3            # trailing stores issued from SP (shorter chain)

_NC = None


def _build_nc():
    nc = bacc.Bacc()
    x_d = nc.declare_dram_parameter("x", [S, C, H + 2, W + 2], F16, isOutput=False)
    wp_d = nc.declare_dram_parameter("wp", [NB, P, 8 * P], F16, isOutput=False)
    se_d = nc.declare_dram_parameter("sebd", [P, P], F16, isOutput=False)
    b_d = nc.declare_dram_parameter("bias", [P, 1], F32, isOutput=False)
    out_d = nc.declare_dram_parameter("out", [S, OC, H, W], F16, isOutput=True)

    with TileContext(nc) as tc:
        with (
            tc.tile_pool(name="constp", bufs=1) as constp,
            tc.tile_pool(name="wstream", bufs=NB) as wstream,
            tc.tile_pool(name="slabp", bufs=1) as slabp,
            tc.tile_pool(name="xpool", bufs=1) as xpool,
            tc.tile_pool(name="outp", bufs=4) as outp,
            tc.tile_pool(name="wgps", bufs=1, space="PSUM") as wgps,
            tc.tile_pool(name="cvps", bufs=1, space="PSUM") as cvps,
        ):
            # --- early DMAs: se via the scalar HWDGE queue so SP's first
            # item (and the DMA line's head) is the first weight block ---
            se_sb = constp.tile([P, P], F16)
            nc.scalar.dma_start(out=se_sb, in_=se_d[:, :])

            xpad = [[None] * CC for _ in range(S)]
            for s in range(S):
                for cc in range(CC):
                    xpad[s][cc] = xpool.tile(
                        [P, H + 2, W + 2], F16, name=f"xpad_{s}_{cc}",
                        tag=f"xpad_{s}_{cc}",
                    )

            def emit_xload(s, cc):
                nc.gpsimd.dma_start(
                    out=xpad[s][cc], in_=x_d[s, cc * P:(cc + 1) * P, :, :],
                )

            # --- weight-bank stream on SP; block 0 lands as two half
            # transfers so the first wgen psum group starts sooner ---
            wtb0 = []
            for j in range(2):
                t = wstream.tile([P, 4, P], F16, name=f"wtb0_{j}",
                                 tag=f"wtb0_{j}")
                nc.sync.dma_start(
                    out=t,
                    in_=wp_d[0, :, j * 4 * P:(j + 1) * 4 * P].rearrange(
                        "p (t c) -> p t c", t=4))
                wtb0.append(t)
            wtbs = [wtb0]
            for b in range(1, NB):
                wtb = wstream.tile([P, 8, P], F16, name=f"wtb_{b}", tag="wtb")
                nc.sync.dma_start(out=wtb, in_=wp_d[b, :, :].rearrange(
                    "p (t c) -> p t c", t=8))
                wtbs.append(wtb)

            # --- x loads (Pool SWDGE queue: descriptor generation runs on
            # the otherwise-idle Pool engine, bypassing the shared HWDGE).
            # tile_wait_until staggers them so the x transfers do not jump
            # ahead of the weight blocks in the DMA-engine line: the tile
            # scheduler reorders ready instructions, so issue order alone
            # would not hold them back. ---
            bias_sb = constp.tile([P, 1], F32)
            nc.scalar.dma_start(out=bias_sb, in_=b_d[:, :])
            for i, s in enumerate(range(NPROG)):
                with tc.tile_wait_until(0.003 + 0.0006 * i):
                    emit_xload(s, 0)
            for i, s in enumerate(range(NPROG)):
                with tc.tile_wait_until(0.007 + 0.001 * i):
                    emit_xload(s, 1)
            for i, s in enumerate(range(NPROG, S)):
                with tc.tile_wait_until(0.011 + 0.001 * i):
                    emit_xload(s, 0)
            for i, s in enumerate(range(NPROG, S)):
                with tc.tile_wait_until(0.016 + 0.001 * i):
                    emit_xload(s, 1)

            # --- warm-up source tile (zeroed so CoreSim sees no NaNs) ---
            zwarm = constp.tile([P, P], F16)
            nc.vector.memset(zwarm, 0.0)

            blocks = [(cc, k) for cc in range(CC) for k in range(KS * KS)]
            cvtile = {}
            warm_idx = [0]

            def emit_warm(tag):
                # dependency-free matmul on the zero tile: soaks PE idle
                # while DMAs are in flight and keeps the p-state ramp hot.
                t = cvps.tile([P, HH, W], F32, name=f"warm_{warm_idx[0]}",
                              tag=f"cps_{tag}")
                warm_idx[0] += 1
                nc.tensor.matmul(
                    t[:, 0:4, :], zwarm, zwarm[:, :4 * W],
                    start=True, stop=True, skip_group_check=True,
                )

            # --- PE warm-up before the first weight block lands ---
            for wi in range(WARM_PRE):
                emit_warm(wi % 6)

            # wgen slab: [c_part, cc, k, s, o] -- conv lhsT slices are
            # wg[:, cc, k, s, :], a contiguous [128, 128] tile.
            wg = slabp.tile([P, CC, KS * KS, S, P], F16)

            def emit_wgen(cc, k, wtb, bi):
                # produce wg[:, cc, k, :, :] (8 o_hi blocks = 2 psum groups)
                split = isinstance(wtb, list)
                for j in range(2):
                    ps = wgps.tile([P, 4 * P], F32, name=f"wgen_{bi}_{j}",
                                   tag=f"wgps_{j}")
                    for i in range(4):
                        stat = (wtb[j][:, i, :] if split
                                else wtb[:, j * 4 + i, :])
                        nc.tensor.matmul(
                            ps[:, i * P:(i + 1) * P], stat,
                            se_sb, start=True, stop=True,
                        )
                    # psum free layout: (o_hi, s, g); slab wants (s, o_hi, g)
                    oh0 = j * 4
                    src = ps.rearrange("p (oh s g) -> p oh s g", oh=4, s=S, g=G)
                    dst = wg[:, cc, k, :, oh0 * G:(oh0 + 4) * G].rearrange(
                        "p s (oh g) -> p oh s g", g=G)
                    eng = nc.vector if (bi < 6 or j == 0) else nc.scalar
                    if eng is nc.vector:
                        eng.tensor_copy(out=dst, in_=src)
                    else:
                        eng.activation(
                            dst, src, mybir.ActivationFunctionType.Identity)

            def conv_psum(s, hi, gi, rows=HH):
                t = cvps.tile([P, rows, W], F32, name=f"cps_{s}_{hi}_{gi}",
                              tag=f"cps_{gi % 6}")
                cvtile[(s, hi)] = t
                return t

            def emit_conv_mm(cc, k, s, hi, h0, rows=HH):
                ky, kx = k // KS, k % KS
                rhs = xpad[s][cc][:, h0 + ky:h0 + ky + rows, kx:kx + W]
                nc.tensor.matmul(
                    cvtile[(s, hi)], wg[:, cc, k, s, :], rhs,
                    start=(cc == 0 and k == 0),
                    stop=(cc == CC - 1 and k == KS * KS - 1),
                    skip_group_check=True,
                )

            store_idx = [0]
            NSTORES = 2 * S + 1  # last sample's hi=1 tile is split in two

            def emit_evac(s, hi, h0, rows=HH):
                ot = outp.tile([P, rows, W], F16,
                               name=f"ot_{s}_{hi}_{h0}", tag=f"ot{rows}")
                nc.scalar.activation(
                    ot, cvtile[(s, hi)],
                    mybir.ActivationFunctionType.Identity,
                    bias=bias_sb[:, 0:1],
                )
                eng = (nc.sync if store_idx[0] >= NSTORES - NSP_STORES
                       else nc.gpsimd)
                store_idx[0] += 1
                eng.dma_start(out=out_d[s, :, h0:h0 + rows, :], in_=ot)

            # --- phase A: stream wgen; progressive samples join as their
            # x tiles land (backfilling earlier blocks), LAG blocks behind ---
            gi = 0
            for s in range(NPROG):
                for hi in range(2):
                    conv_psum(s, hi, gi)
                    gi += 1

            def conv_block_emit(cb):
                for s in range(NPROG):
                    if cb < JOIN[s]:
                        continue
                    rng = range(cb + 1) if cb == JOIN[s] else (cb,)
                    for b2 in rng:
                        cc2, k2 = blocks[b2]
                        for hi in range(2):
                            emit_conv_mm(cc2, k2, s, hi, hi * HH)

            for bi, (cc, k) in enumerate(blocks):
                emit_wgen(cc, k, wtbs[bi], bi)
                if bi < len(FILLS):
                    for _ in range(FILLS[bi]):
                        emit_warm(4 + warm_idx[0] % 2)
                cb = bi - LAG
                if cb >= 0:
                    conv_block_emit(cb)
            for cb in range(NB - LAG, NB):
                conv_block_emit(cb)
            for s in range(NPROG):
                for hi in range(2):
                    emit_evac(s, hi, hi * HH)

            # --- phase B: dense bursts for the remaining samples; the very
            # last output tile runs as two half-height groups so the final
            # evac+store chain is shorter ---
            for s in range(NPROG, S):
                for hi in range(2):
                    if s == S - 1 and hi == 1:
                        for half in range(2):
                            h0 = hi * HH + half * (HH // 2)
                            conv_psum(s, hi, gi, rows=HH // 2)
                            gi += 1
                            for cc, k in blocks:
                                emit_conv_mm(cc, k, s, hi, h0, rows=HH // 2)
                            emit_evac(s, hi, h0, rows=HH // 2)
                    else:
                        conv_psum(s, hi, gi)
                        gi += 1
                        for cc, k in blocks:
                            emit_conv_mm(cc, k, s, hi, hi * HH)
                        emit_evac(s, hi, hi * HH)

    nc.compile()
    return nc


def _get_nc():
    global _NC
    if _NC is None:
        _NC = _build_nc()
    return _NC


def _prep_core_inputs(inputs, inputs_se, weight, bias, bg, oh):
    # weight rows: r = o*(C*9) + c*9 + (ky*3+kx)  -> [O, C, 3, 3, NUM]
    wr = weight.reshape(O, C, KS, KS, NUM)
    wo = wr[oh * OC:(oh + 1) * OC]            # [128, 256, 3, 3, 8]
    p_arr = np.arange(NP)
    k_arr = p_arr // OC                       # k index per (m,g) pair
    o_arr = p_arr % OC
    t = wo[o_arr, :, k_arr // KS, k_arr % KS, :]     # [1152, 256, 8]
    wp = (
        t.reshape(NM, G, CC, P, NUM)
        .transpose(2, 0, 4, 1, 3)             # cc, m, n, g, c
        .reshape(CC * NM, P, P)
    )
    # regroup into [NB, P, 8*P]: block (cc,k) holds its 8 stationary tiles
    # pre-transposed so each DRAM partition line is 2048 contiguous bytes
    wp = (
        wp.reshape(CC, KS * KS, 8, P, P)
        .transpose(0, 1, 3, 2, 4)             # cc, k, p, t, c
        .reshape(NB, P, 8 * P)
    )
    wp = np.ascontiguousarray(wp.astype(np.float16))

    se_core = inputs_se[bg * S:(bg + 1) * S]  # [8, 8] (s, n)
    sebd = np.zeros((NUM, G, S, G), dtype=np.float32)
    for g in range(G):
        sebd[:, g, :, g] = se_core.T
    sebd = sebd.reshape(P, P).astype(np.float16)

    x_core = np.pad(
        inputs[bg * S:(bg + 1) * S], ((0, 0), (0, 0), (1, 1), (1, 1))
    )
    return {
        "x": np.ascontiguousarray(x_core.astype(np.float16)),
        "wp": wp,
        "sebd": sebd,
        "bias": np.ascontiguousarray(
            bias[oh * OC:(oh + 1) * OC].reshape(OC, 1), dtype=np.float32
        ),
    }


def kernel(inputs, inputs_se, weight, bias):
    inputs = np.asarray(inputs, dtype=np.float32)
    inputs_se = np.asarray(inputs_se, dtype=np.float32)
    weight = np.asarray(weight, dtype=np.float32)
    bias = np.asarray(bias, dtype=np.float32)

    nc = _get_nc()
    in_maps = []
    for core in range(NCORES):
        bg, oh = core // OHALF, core % OHALF
        in_maps.append(_prep_core_inputs(inputs, inputs_se, weight, bias, bg, oh))

    res = run_bass_kernel_spmd(nc, in_maps, list(range(NCORES))).results

    out = np.empty((B, O, H, W), dtype=np.float32)
    for core in range(NCORES):
        bg, oh = core // OHALF, core % OHALF
        out[bg * S:(bg + 1) * S, oh * OC:(oh + 1) * OC] = (
            res[core]["out"].astype(np.float32))
    return out
